# revision 12
# baseline (speedup 1.0000x reference)
"""Multi-head attention (B=2, S=2048, D=1024, H=16, d_head=64) on 8 TRN2 cores.

Sharding: 2-way data parallel over batch x 4-way tensor parallel over heads.
Core c: batch g = c//4, heads [4r, 4r+4) with r = c%4.

v5: fused pipeline with a fast ramp and per-(q4, ep) AllGather rounds.

  - Ramp: weights pre-packed on host for 4KB-contiguous descriptors;
    xq/xv loaded as whole [128, 2048] rows into persistent SBUF (no pool
    recycling -> the DMA FIFO never stalls); K proj -> Q proj(q4=0, et=0)
    -> sweep 0.  A dummy EXP preloads the activation table set.
  - V proj runs inside sweep 0 (PV lag 6 there, 2 elsewhere).
  - Each (q4, ep) flush normalizes the PV accumulator into a [128, 512]
    staging pair (lh0 rows 0:64 direct, lh1 via a second tile), DMAs it
    to DRAM and fires an ep-split AllGather [128,512] -> [512,512].  The
    CC stream is flush-paced (8 x ~18us in ~20us gaps), so only the last
    AG is exposed in the tail.
  - Wout chunks are in-loop aux work reading gathered slabs (Tile deps on
    the AG output DMAs; no manual semaphores); Wout(3) runs in the tail
    with the even/odd slab interleave so its first half hides under the
    final AllGather.
"""

import os
import sys

import numpy as np

for _p in ("/opt/trn_rl_repo",):
    if _p not in sys.path and os.path.isdir(_p):
        sys.path.append(_p)

import ml_dtypes

import concourse.bacc as bacc
import concourse.mybir as mybir
from concourse.bass_utils import run_bass_kernel_spmd
from concourse.tile import TileContext

P = 128
B, S, DM = 2, 2048, 1024
NH_TOT, EH = 16, 64
NCORES = 8
GROUPS = 2
NH = 4  # heads per core
EHC = NH * EH  # 256
NDT = DM // P  # 8
NKT = S // P  # 16
QC = 512
NQC = S // QC  # 4
VW = EH + 1  # V width incl. ones column

BF = mybir.dt.bfloat16
F32 = mybir.dt.float32

_cached_nc = None


def voff(kt, h):
    return (kt * NH + h) * VW


def build_nc():
    nc = bacc.Bacc("TRN2", target_bir_lowering=False, debug=False, num_devices=NCORES)

    xqt = nc.declare_dram_parameter("xqt", [DM, S], BF, isOutput=False)
    xkt = nc.declare_dram_parameter("xkt", [DM, S], BF, isOutput=False)
    xvt = nc.declare_dram_parameter("xvt", [DM, S], BF, isOutput=False)
    # weights pre-packed on host: [P, NDT*EHC], partition-contiguous
    wqt = nc.declare_dram_parameter("wqt", [P, NDT * EHC], BF, isOutput=False)
    wkt = nc.declare_dram_parameter("wkt", [P, NDT * EHC], BF, isOutput=False)
    wvt = nc.declare_dram_parameter("wvt", [P, NDT * EHC], BF, isOutput=False)
    wot = nc.declare_dram_parameter("wot", [P, NDT * EHC], BF, isOutput=False)
    outt = nc.declare_dram_parameter("outt", [EHC, S], F32, isOutput=True)

    with TileContext(nc) as tc:
        with (
            tc.tile_pool(name="persist", bufs=1) as persist,
            tc.tile_pool(name="dram", bufs=1, space="DRAM") as dram,
        ):
            # --- persistent SBUF ---
            wq_sb = persist.tile([P, NDT, EHC], BF)
            wk_sb = persist.tile([P, NDT, EHC], BF)
            wv_sb = persist.tile([P, NDT, EHC], BF)
            wo_sb = persist.tile([P, NDT, EHC], BF)
            for wsb, wpar in ((wk_sb, wkt), (wq_sb, wqt), (wv_sb, wvt), (wo_sb, wot)):
                nc.gpsimd.dma_start(
                    wsb[:].rearrange("p dt e -> p (dt e)"), wpar[:, :]
                )

            qt_sb = [persist.tile([P, S], BF, name=f"qt{et}") for et in range(2)]
            kt_sb = [persist.tile([P, S], BF, name=f"kt{et}") for et in range(2)]
            v_sb = persist.tile([P, NKT * NH * VW + P - VW], BF)
            nc.gpsimd.memset(v_sb[:], 1.0)  # ones columns; V data overwrites 0:64
            ones_sb = persist.tile([P, EH], BF)
            nc.gpsimd.memset(ones_sb[:], 1.0)
            xq_sb = persist.tile([P, NDT, S], BF)
            xv_sb = persist.tile([P, NDT, S], BF)
            xk_t = [persist.tile([P, S], BF, name=f"xk{dt}") for dt in range(NDT)]

            # exp-table preload: tiny dummy activation during the ramp
            dume = persist.tile([P, 8], F32)
            nc.gpsimd.memset(dume[0:1, :], 0.0)
            nc.scalar.activation(
                dume[0:1, 0:8], dume[0:1, 0:8], mybir.ActivationFunctionType.Exp
            )

            # per-round DRAM staging + gathered buffers
            hloc = [dram.tile([P, QC], BF, name=f"hloc{r}") for r in range(8)]
            hgat = [dram.tile([4 * P, QC], BF, name=f"hgat{r}") for r in range(8)]

            def emit_allgather(r):
                nc.gpsimd.collective_compute(
                    "AllGather",
                    mybir.AluOpType.bypass,
                    replica_groups=[[0, 1, 2, 3], [4, 5, 6, 7]],
                    ins=[hloc[r].opt()],
                    outs=[hgat[r].opt()],
                )

            # --- input DMA order (sync queue): xk, xq[0:512], xv, xq rest ---
            for dt in range(NDT):
                nc.sync.dma_start(xk_t[dt][:], xkt[dt * P : (dt + 1) * P, :])
            for dt in range(NDT):
                nc.sync.dma_start(
                    xq_sb[:, dt, 0:QC], xqt[dt * P : (dt + 1) * P, 0:QC]
                )
            for dt in range(NDT):
                nc.sync.dma_start(
                    xv_sb[:, dt, 0:1024], xvt[dt * P : (dt + 1) * P, 0:1024]
                )
            for dt in range(NDT):
                nc.sync.dma_start(
                    xv_sb[:, dt, 1024:S], xvt[dt * P : (dt + 1) * P, 1024:S]
                )
            for dt in range(NDT):
                nc.sync.dma_start(
                    xq_sb[:, dt, QC:S], xqt[dt * P : (dt + 1) * P, QC:S]
                )

            # --- K proj (dt-outer, 8 PSUM banks) ---
            with tc.tile_pool(name="projk", bufs=1, space="PSUM") as projk:
                kps = [
                    [
                        projk.tile([P, QC], F32, name=f"kp{et}_{qc}")
                        for qc in range(NQC)
                    ]
                    for et in range(2)
                ]
                for dt in range(NDT):
                    for et in range(2):
                        for qc in range(NQC):
                            nc.tensor.matmul(
                                kps[et][qc][:],
                                wk_sb[:, dt, et * P : (et + 1) * P],
                                xk_t[dt][:, qc * QC : (qc + 1) * QC],
                                start=(dt == 0),
                                stop=(dt == NDT - 1),
                            )
                for et in range(2):
                    for qc in range(NQC):
                        nc.vector.tensor_copy(
                            kt_sb[et][:, qc * QC : (qc + 1) * QC], kps[et][qc][:]
                        )

            # --- Q proj q4=0, et=0 only (needed by sweep 0) ---
            with tc.tile_pool(name="projq", bufs=1, space="PSUM") as projq:
                qp0 = projq.tile([P, QC], F32, name="qp0")
                for dt in range(NDT):
                    nc.tensor.matmul(
                        qp0[:],
                        wq_sb[:, dt, 0:P],
                        xq_sb[:, dt, 0:QC],
                        start=(dt == 0),
                        stop=(dt == NDT - 1),
                    )
                nc.vector.tensor_copy(qt_sb[0][:, 0:QC], qp0[:])

            # --- fused attention + aux pipeline ---
            with (
                tc.tile_pool(name="scorep", bufs=2, space="PSUM") as scorep,
                tc.tile_pool(name="pvp", bufs=1, space="PSUM") as pvp,
                tc.tile_pool(name="auxp", bufs=2, space="PSUM") as auxp,
                tc.tile_pool(name="expp", bufs=8) as expp,
                tc.tile_pool(name="pvdp", bufs=1) as pvdp,
                tc.tile_pool(name="rcpp", bufs=1) as rcpp,
                tc.tile_pool(name="stgp", bufs=2) as stgp,
                tc.tile_pool(name="hallp", bufs=2) as hallp,
                tc.tile_pool(name="outsp", bufs=2) as outsp,
            ):
                hall_t = {}  # q4 -> [8 slab tiles]

                def emit_hall(q4, dts):
                    """Load gathered slabs for quarter q4 into SBUF.
                    Slab dt holds canonical head dims [128*dt, 128*(dt+1)):
                    source rank dt//2, ep dt%2."""
                    if q4 not in hall_t:
                        hall_t[q4] = [None] * NDT
                    for dt in dts:
                        t = hallp.tile([P, QC], BF, name=f"hl{dt}", tag=f"hl{dt}")
                        hall_t[q4][dt] = t
                        src = hgat[2 * q4 + (dt % 2)][
                            (dt // 2) * P : (dt // 2 + 1) * P, :
                        ]
                        nc.sync.dma_start(t[:], src)

                # --- aux chunk emitters ---
                def vproj_chunk(tt):
                    def go():
                        psv = auxp.tile([P, QC], F32, name="aux", tag="aux")
                        for dt in range(NDT):
                            nc.tensor.matmul(
                                psv[:, 0:EHC],
                                xv_sb[:, dt, tt * P : (tt + 1) * P],
                                wv_sb[:, dt, :],
                                start=(dt == 0),
                                stop=(dt == NDT - 1),
                                skip_group_check=True,
                            )
                        nc.vector.tensor_copy(
                            v_sb[:, tt * NH * VW : (tt + 1) * NH * VW].rearrange(
                                "p (h w) -> p h w", w=VW
                            )[:, :, 0:EH],
                            psv[:, 0:EHC].rearrange("p (h e) -> p h e", e=EH),
                        )

                    return [go]

                def qproj_chunk(q4, et):
                    state = {}

                    def part(dts, last):
                        def go():
                            if "ps" not in state:
                                state["ps"] = auxp.tile(
                                    [P, QC], F32, name="aux", tag="aux"
                                )
                            ps = state["ps"]
                            for dt in dts:
                                nc.tensor.matmul(
                                    ps[:],
                                    wq_sb[:, dt, et * P : (et + 1) * P],
                                    xq_sb[:, dt, q4 * QC : (q4 + 1) * QC],
                                    start=(dt == 0),
                                    stop=(dt == NDT - 1),
                                    skip_group_check=True,
                                )
                            if last:
                                nc.vector.tensor_copy(
                                    qt_sb[et][:, q4 * QC : (q4 + 1) * QC], ps[:]
                                )

                        return go

                    return [part(range(0, 4), False), part(range(4, NDT), True)]

                def wout_chunk(q4, ot, order=None):
                    state = {}
                    js = list(order) if order else list(range(NDT))

                    def part(jss, last):
                        def go():
                            if "ps" not in state:
                                state["ps"] = auxp.tile(
                                    [P, QC], F32, name="aux", tag="aux"
                                )
                            ps = state["ps"]
                            for i, j in enumerate(jss):
                                first = j == js[0]
                                lastj = j == js[-1]
                                nc.tensor.matmul(
                                    ps[:],
                                    wo_sb[:, j, ot * P : (ot + 1) * P],
                                    hall_t[q4][j][:],
                                    start=first,
                                    stop=lastj,
                                    skip_group_check=True,
                                )
                            if last:
                                ob = outsp.tile([P, QC], F32, name="ob", tag="ob")
                                nc.vector.tensor_copy(ob[:], ps[:])
                                nc.gpsimd.dma_start(
                                    outt[
                                        ot * P : (ot + 1) * P,
                                        q4 * QC : (q4 + 1) * QC,
                                    ],
                                    ob[:],
                                )

                        return go

                    half = len(js) // 2
                    return [part(js[:half], False), part(js[half:], True)]

                # --- sweep-end normalize + stage + gather ---
                def flush_a(pvt):
                    pvd = pvdp.tile([P, 2 * QC], F32, name="pvd", tag="pvd")
                    nc.vector.tensor_copy(pvd[0 : EH + 1, :], pvt[0 : EH + 1, :])
                    rcp = rcpp.tile([P, 2 * QC], F32, name="rcp", tag="rcpf")
                    # custom DVE op mislowers at base_partition>0: run on
                    # rows 0:65; only row 64 (denominator) is read after.
                    nc.vector.reciprocal_approx_fast(
                        rcp[0 : EH + 1, :], pvd[0 : EH + 1, :]
                    )
                    rcpb = rcpp.tile([P, 2 * QC], BF, name="rcpb", tag="rcpb")
                    nc.vector.tensor_copy(rcpb[EH : EH + 1, :], rcp[EH : EH + 1, :])
                    return pvd, rcpb

                def flush_b(r, fstate):
                    pvd, rcpb = fstate
                    bc = auxp.tile([P, QC], F32, name="bc", tag="aux")
                    nc.tensor.matmul(
                        bc[0:EH, :],
                        ones_sb[EH : EH + 1, 0:EH],
                        rcpb[EH : EH + 1, 0:QC],
                        start=True,
                        stop=True,
                    )
                    nc.tensor.matmul(
                        bc[EH : 2 * EH, :],
                        ones_sb[EH : EH + 1, 0:EH],
                        rcpb[EH : EH + 1, QC : 2 * QC],
                        start=True,
                        stop=True,
                        skip_group_check=True,
                    )
                    for lh in range(2):
                        stg = stgp.tile([P, QC], BF, name="stg", tag=f"stg{lh}")
                        nc.vector.tensor_mul(
                            stg[0:EH, :],
                            pvd[0:EH, lh * QC : (lh + 1) * QC],
                            bc[lh * EH : (lh + 1) * EH, :],
                        )
                        nc.gpsimd.dma_start(
                            hloc[r][lh * EH : (lh + 1) * EH, :], stg[0:EH, :]
                        )
                    emit_allgather(r)

                # per-sweep aux step schedule: {kt: [steps]}
                def sweep_steps(s):
                    steps = {}

                    def put2(kts, chunk):
                        for kt, st in zip(kts, chunk):
                            steps.setdefault(kt, []).append(st)

                    if s == 0:
                        put2((1, 2), qproj_chunk(0, 1))
                        for tt in range(NKT):
                            steps.setdefault(tt + 3, []).extend(vproj_chunk(tt))
                    elif s == 1:
                        put2((2, 4), qproj_chunk(1, 0))
                        put2((8, 10), qproj_chunk(1, 1))
                    elif s == 2:
                        put2((4, 6), qproj_chunk(2, 0))
                        put2((10, 12), qproj_chunk(2, 1))
                    elif s == 3:
                        put2((2, 4), qproj_chunk(3, 0))
                        put2((8, 10), qproj_chunk(3, 1))
                        steps.setdefault(12, []).append(lambda: emit_hall(0, range(NDT)))
                    elif s == 4:
                        put2((2, 4), wout_chunk(0, 0))
                        put2((8, 10), wout_chunk(0, 1))
                    elif s == 5:
                        steps.setdefault(10, []).append(lambda: emit_hall(1, range(NDT)))
                    elif s == 6:
                        steps.setdefault(12, []).append(
                            lambda: emit_hall(2, (0, 2, 4, 6))
                        )
                    elif s == 7:
                        steps.setdefault(2, []).append(
                            lambda: emit_hall(2, (1, 3, 5, 7))
                        )
                    return steps

                pending = None  # (round, pvt) awaiting normalize
                for q4 in range(NQC):
                    q0 = q4 * QC
                    for ep in range(2):
                        s = q4 * 2 + ep
                        lag = 6 if s == 0 else 2
                        steps = sweep_steps(s)
                        pvt = pvp.tile([P, 2 * QC], F32, name="pv", tag="pv")
                        exring = [None] * NKT
                        for kt in range(NKT + lag):
                            if kt < NKT:
                                exq = expp.tile([P, 1024], BF, name="exq", tag="exq")
                                exring[kt] = exq
                                s_t = scorep.tile([P, 1024], F32, name="sq", tag="sq")
                                for lh in range(2):
                                    po = lh * EH
                                    nc.tensor.matmul(
                                        s_t[:, lh * QC : (lh + 1) * QC],
                                        kt_sb[ep][
                                            po : po + EH, kt * P : (kt + 1) * P
                                        ],
                                        qt_sb[ep][po : po + EH, q0 : q0 + QC],
                                        start=True,
                                        stop=True,
                                    )
                                nc.scalar.activation(
                                    exq[:],
                                    s_t[:],
                                    mybir.ActivationFunctionType.Exp,
                                    scale=float(1.0 / np.sqrt(EH)),
                                )
                            if kt == 1 and pending is not None:
                                fstate = flush_a(pending[1])
                            if kt == 4 and pending is not None:
                                flush_b(pending[0], fstate)
                                pending = None
                            if kt >= lag:
                                pkt = kt - lag
                                for lh in range(2):
                                    h = 2 * ep + lh
                                    nc.tensor.matmul(
                                        pvt[:, lh * QC : (lh + 1) * QC],
                                        v_sb[:, voff(pkt, h) : voff(pkt, h) + P],
                                        exring[pkt][:, lh * QC : (lh + 1) * QC],
                                        start=(pkt == 0),
                                        stop=(pkt == NKT - 1),
                                        skip_group_check=True,
                                    )
                            for st in steps.get(kt, ()):
                                st()
                        pending = (s, pvt)

                # --- tail: flush (3,1) fires the last AG; deferred Wout(1),
                # Wout(2), Wout(3)-even fill the PE while it runs ---
                fstate = flush_a(pending[1])
                flush_b(pending[0], fstate)
                emit_hall(3, (0, 2, 4, 6))
                for st in wout_chunk(1, 0):
                    st()
                for st in wout_chunk(1, 1):
                    st()
                for st in wout_chunk(2, 0):
                    st()
                for st in wout_chunk(2, 1):
                    st()
                wc30 = wout_chunk(3, 0, order=(0, 2, 4, 6, 1, 3, 5, 7))
                wc31 = wout_chunk(3, 1, order=(0, 2, 4, 6, 1, 3, 5, 7))
                wc30[0]()
                wc31[0]()
                emit_hall(3, (1, 3, 5, 7))
                wc30[1]()
                wc31[1]()

    nc.compile()
    return nc


def _prep_inputs(x_query, x_key, x_value, Wq, Wk, Wv, Wout):
    bf = ml_dtypes.bfloat16
    xt = {}
    for g in range(GROUPS):
        xt[g] = tuple(
            np.ascontiguousarray(np.asarray(x[g], dtype=np.float32).T).astype(bf)
            for x in (x_query, x_key, x_value)
        )

    def pack_w(w2d):  # [out(EHC), in(DM)] -> [P, NDT*out] partition-packed
        wt = np.ascontiguousarray(np.asarray(w2d, dtype=np.float32).reshape(-1, DM).T)
        n_out = wt.shape[1]
        return np.ascontiguousarray(
            wt.reshape(NDT, P, n_out).transpose(1, 0, 2).reshape(P, NDT * n_out)
        ).astype(bf)

    in_maps = []
    wof = np.asarray(Wout, dtype=np.float32)
    for c in range(NCORES):
        g, r = c // 4, c % 4
        hs = slice(NH * r, NH * (r + 1))
        in_maps.append(
            {
                "xqt": xt[g][0],
                "xkt": xt[g][1],
                "xvt": xt[g][2],
                "wqt": pack_w(Wq[hs]),
                "wkt": pack_w(Wk[hs]),
                "wvt": pack_w(Wv[hs]),
                "wot": pack_w(wof[EHC * r : EHC * (r + 1), :]),
            }
        )
    return in_maps


def kernel(x_query, x_key, x_value, Wq, Wk, Wv, Wout, _trace=False):
    global _cached_nc
    if _cached_nc is None:
        _cached_nc = build_nc()
    nc = _cached_nc

    in_maps = _prep_inputs(x_query, x_key, x_value, Wq, Wk, Wv, Wout)
    res = run_bass_kernel_spmd(nc, in_maps, list(range(NCORES)), trace=_trace)
    kernel.last_result = res

    out = np.empty((B, S, DM), dtype=np.float32)
    for c in range(NCORES):
        g, r = c // 4, c % 4
        out[g, :, EHC * r : EHC * (r + 1)] = res.results[c]["outt"].T
    return out
